# revision 10
# baseline (speedup 1.0000x reference)
"""Trainium2 Bass kernel for nn_GCBlock9 (dynamic depthwise smoothing + 9-neighbor
sigmoid diffusion block).

Contract: kernel(**inputs) takes the FULL unsharded inputs (as from
setup_inputs()) and returns the full output tuple (q, y) matching the
reference. Self-contained: shapes/sharding hardcoded.

Distribution: the 512 (b, c) planes are data-parallel; 64 planes per core on 8
cores. On-core layout: 128 partitions = (half, plane) with each partition
holding a 48-row half-plane in padded buffers so every conv tap / 9-neighbor
shift is a pure free-dim AP offset (halo rows recomputed redundantly).

The tiny dense chain (GAP -> 1x1 convs -> 27 per-plane filter scalars, and the
q output head) couples all 256 channels, so it is split off: launch A reduces
each plane on-device, the filter scalars are formed on host (f64), and launch B
does the heavy per-plane work:
  - TensorE: all 4 depthwise convs as diagonal-matmul PSUM accumulations
    (per-plane weights on the diagonal, shifts as rhs AP offsets), fp32.
  - VectorE: fused scalar_tensor_tensor MACs, the fp16 sigmoid-diffusion chain,
    reciprocal, final combine.
  - ScalarE: Square/Exp/Sigmoid LUTs + PSUM evictions.
"""

import json

import numpy as np

import concourse.bass as bass
import concourse.mybir as mybir
from concourse.alu_op_type import AluOpType
from concourse.bass_utils import run_bass_kernel_spmd
from concourse.tile import TileContext

# The walrus build in this container accepts at most one sync-wait command per
# instruction; Tile emits several (one per producer semaphore). Hoist the
# excess onto standalone EventSemaphore waits (same engine, immediately before
# the consumer — exactly what raw-bass wait_ge() emits), via a to_json_bytes
# post-pass so both compile paths (direct + bass2jax/PJRT) are covered.
_MAX_WAITS = 1


def _split_sync_waits(js: bytes) -> bytes:
    data = json.loads(js)
    n = 0
    for f in data.get("functions", []):
        for bb in f.get("blocks", []):
            out = []
            for ins in bb.get("instructions", []):
                si = ins.get("sync_info")
                if si:
                    w = si.get("on_wait") or []
                    if len(w) > _MAX_WAITS:
                        excess, keep = w[:-_MAX_WAITS], w[-_MAX_WAITS:]
                        for x in excess:
                            n += 1
                            out.append({
                                "debug": ins.get("debug"),
                                "engine": ins["engine"],
                                "ins": [], "outs": [],
                                "name": f"{ins['name']}_hw{n}",
                                "opcode": "EventSemaphore",
                                "sync_info": {"on_update": [], "on_wait": [x]}})
                        si["on_wait"] = keep
                out.append(ins)
            bb["instructions"] = out
    return json.dumps(data).encode()


if not getattr(bass.Bass, "_wait_split_installed", False):
    _orig_to_json = bass.Bass.to_json_bytes
    bass.Bass.to_json_bytes = (
        lambda self, *a, **k: _split_sync_waits(_orig_to_json(self, *a, **k)))
    bass.Bass._wait_split_installed = True

F32 = mybir.dt.float32
F16 = mybir.dt.float16
AF = mybir.ActivationFunctionType

N_CORES = 8
B, C, H, W = 2, 256, 96, 96
NPLANES = B * C                 # 512
PPC = NPLANES // N_CORES        # 64 planes per core
HH = H // 2                     # 48 rows per half-plane

N_DIAG = 30                     # 9 f9 + 9 |e9| + 9 e9 + 3 wd
TAB_COLS = 16                   # w9 (center pre-halved) in cols 0..8

_BUILD_CACHE = {}


# --------------------------------------------------------------------------- #
# host-side dense chain (0.1% of FLOPs; couples all channels)
# --------------------------------------------------------------------------- #

def _gate_chain_host(inputs, gap):
    dt = np.float64
    q = np.asarray(inputs['query'], dt)                      # (100,B,C)
    w_dp = np.asarray(inputs['w_dp'], dt); b_dp = np.asarray(inputs['b_dp'], dt)
    bn_g = np.asarray(inputs['bn_g'], dt); bn_b = np.asarray(inputs['bn_b'], dt)
    bn_rm = np.asarray(inputs['bn_rm'], dt); bn_rv = np.asarray(inputs['bn_rv'], dt)
    w_dc = np.asarray(inputs['w_dc'], dt); b_dc = np.asarray(inputs['b_dc'], dt)
    w_lin = np.asarray(inputs['w_lin'], dt); b_lin = np.asarray(inputs['b_lin'], dt)
    w_delin = np.asarray(inputs['w_delin'], dt); b_delin = np.asarray(inputs['b_delin'], dt)

    x = gap.astype(dt) @ w_dp.T + b_dp                       # (B,4C)
    x = (x - bn_rm) / np.sqrt(bn_rv + 1e-5) * bn_g + bn_b
    x = np.maximum(x, 0)
    dyn = 1.0 / (1.0 + np.exp(-(x @ w_dc.T + b_dc)))         # (B,3C)
    d_f, d_w, d_e = np.split(dyn, 3, axis=-1)                # (B,C)
    lin = np.einsum('qbc,kq->bck', q, w_lin) + b_lin         # (B,C,27)
    f9 = lin[..., 0:9] * d_f[..., None]
    w9 = lin[..., 9:18] * d_w[..., None]
    e9 = lin[..., 18:27] * d_e[..., None]
    qcat = np.concatenate([f9, w9, e9], axis=-1)             # (B,C,27)
    qout = (qcat @ w_delin.T + b_delin).transpose(2, 0, 1)   # (100,B,C)
    return (f9.reshape(NPLANES, 9).astype(np.float32),
            w9.reshape(NPLANES, 9).astype(np.float32),
            e9.reshape(NPLANES, 9).astype(np.float32),
            qout.astype(np.float32))


# --------------------------------------------------------------------------- #
# launch A: per-plane sums (for GAP)
# --------------------------------------------------------------------------- #

def _build_gap_nc():
    nc = bass.Bass()
    xv = nc.dram_tensor("xv", [PPC, H, W], F32, kind="ExternalInput")
    gs = nc.dram_tensor("gsum", [128, 1], F32, kind="ExternalOutput")
    with TileContext(nc) as tc:
        with tc.tile_pool(name="p", bufs=1) as pool:
            g = pool.tile([128, HH * W], F32)
            s = pool.tile([128, 1], F32)
            nc.sync.dma_start(out=g[0:PPC, :], in_=xv[:, 0:HH, :])
            nc.sync.dma_start(out=g[PPC:128, :], in_=xv[:, HH:H, :])
            nc.vector.tensor_reduce(
                out=s[0:PPC, :], in_=g[0:PPC, :], axis=mybir.AxisListType.X,
                op=AluOpType.add)
            nc.vector.tensor_reduce(
                out=s[PPC:128, :], in_=g[PPC:128, :], axis=mybir.AxisListType.X,
                op=AluOpType.add)
            nc.sync.dma_start(out=gs[:, :], in_=s[:, :])
    return nc


# --------------------------------------------------------------------------- #
# launch B: the per-plane heavy math
# --------------------------------------------------------------------------- #

def _build_main_nc():
    nc = bass.Bass()
    xv = nc.dram_tensor("xv", [PPC, H, W], F32, kind="ExternalInput")
    diags = nc.dram_tensor("diags", [N_DIAG, 128, 128], F32, kind="ExternalInput")
    tabd = nc.dram_tensor("tab", [128, TAB_COLS], F32, kind="ExternalInput")
    y_out = nc.dram_tensor("y", [PPC, H, W], F32, kind="ExternalOutput")

    with TileContext(nc) as tc:
        with tc.tile_pool(name="sb", bufs=1) as pool, \
             tc.tile_pool(name="ps", bufs=4, space="PSUM") as psum:

            V52 = pool.tile([128, 52, 98], F32)       # padded value, 2-row halo
            V16 = pool.tile([128, 52, 98], F16)
            YT = pool.tile([128, 50, 96], F32)        # ytemp (rows: plane r-1)
            D52 = pool.tile([128, 50, 98], F32)       # diff, padded, 1-row halo
            VD52 = pool.tile([128, 50, 98], F32)      # value*diff, padded
            VD16 = pool.tile([128, 50, 98], F16)
            DEN = pool.tile([128, 48, 96], F32, tag="V52")   # reuse V52 memory
            RCP = pool.tile([128, 48, 96], F32, tag="D52")   # reuse D52 memory
            RD = pool.tile([128, 48, 96], F32)
            NUM = pool.tile([128, 48, 96], F32)
            Y2 = pool.tile([128, 48, 96], F32, tag="RD")     # reuse RD memory
            Y1 = pool.tile([128, 48, 96], F16)
            YO = pool.tile([128, 48, 96], F32)
            TAB = pool.tile([128, TAB_COLS], F32)
            DG = pool.tile([128, N_DIAG, 128], F32)
            EPS = pool.tile([128, 1], F32)
            nc.vector.memset(EPS[:, :], 1e-10)

            # ---- loads + padding zeros ----
            nc.sync.dma_start(out=TAB[:, :], in_=tabd[:, :])
            nc.sync.dma_start(out=DG[:, :, :], in_=diags.rearrange("d p m -> p d m"))
            nc.vector.memset(V52[0:64, 0:2, :], 0.0)       # plane rows -2,-1
            nc.vector.memset(V52[64:128, 50:52, :], 0.0)   # plane rows 96,97
            nc.vector.memset(V52[:, :, 0:1], 0.0)
            nc.vector.memset(V52[:, :, 97:98], 0.0)
            nc.vector.memset(D52[:, :, 0:1], 0.0)
            nc.vector.memset(D52[:, :, 97:98], 0.0)
            nc.vector.memset(VD52[:, :, 0:1], 0.0)
            nc.vector.memset(VD52[:, :, 97:98], 0.0)
            # half0 covers plane rows -2..49, half1 covers 46..97
            nc.sync.dma_start(out=V52[0:64, 2:52, 1:97], in_=xv[:, 0:50, :])
            nc.sync.dma_start(out=V52[64:128, 0:50, 1:97], in_=xv[:, 46:96, :])

            def dg(i):
                return DG[:, i, :]

            # ---- phase A: ytemp = conv3x3(V, f9) on TensorE (10 x 5-row chunks)
            for c in range(10):
                pt = psum.tile([128, 5, 96], F32, tag="pA")
                for k in range(9):
                    di, dj = divmod(k, 3)
                    nc.tensor.matmul(
                        pt[:, :, :], dg(k),
                        V52[:, di + 5 * c: di + 5 * c + 5, dj: dj + 96],
                        start=(k == 0), stop=(k == 8))
                nc.scalar.copy(YT[:, 5 * c: 5 * c + 5, :], pt[:, :, :])

            # D = exp(-(V - ytemp)^2) ; VD = V * D   (rows = plane r-1, 50 rows)
            nc.vector.tensor_sub(D52[:, :, 1:97], V52[:, 1:51, 1:97], YT[:, :, :])
            nc.scalar.activation(D52[:, :, 1:97], D52[:, :, 1:97], AF.Square)
            nc.scalar.activation(D52[:, :, 1:97], D52[:, :, 1:97], AF.Exp, scale=-1.0)
            # plane-boundary halo rows of D must be zero (zero-pad semantics)
            nc.vector.memset(D52[0:64, 0:1, 1:97], 0.0)
            nc.vector.memset(D52[64:128, 49:50, 1:97], 0.0)
            nc.vector.tensor_mul(VD52[:, :, 1:97], V52[:, 1:51, 1:97], D52[:, :, 1:97])
            nc.vector.tensor_copy(out=VD16[:, :, :], in_=VD52[:, :, :])
            nc.vector.tensor_copy(out=V16[:, :, :], in_=V52[:, :, :])

            # ---- phase B convs on TensorE (12 x 4-row chunks each) ----
            # yd9 = conv3x3(D, |e9|) -> DEN = yd9 + 1e-10 (fused in eviction)
            for c in range(12):
                pt = psum.tile([128, 4, 96], F32, tag="pB")
                for k in range(9):
                    di, dj = divmod(k, 3)
                    nc.tensor.matmul(
                        pt[:, :, :], dg(9 + k),
                        D52[:, di + 4 * c: di + 4 * c + 4, dj: dj + 96],
                        start=(k == 0), stop=(k == 8))
                nc.scalar.activation(DEN[:, 4 * c: 4 * c + 4, :], pt[:, :, :],
                                     AF.Identity, bias=EPS[:, 0:1])
            # 1/den as exp(-ln(den)): both LUTs live in the same ACT table set
            # as Square/Exp (natural_log_exp_and_others), ~1e-5 rel err.
            nc.scalar.activation(RCP[:, :, :], DEN[:, :, :], AF.Ln)
            nc.scalar.activation(RCP[:, :, :], RCP[:, :, :], AF.Exp, scale=-1.0)

            # rd = conv1x3(VD, wd)
            for c in range(12):
                pt = psum.tile([128, 4, 96], F32, tag="pB")
                for j in range(3):
                    nc.tensor.matmul(
                        pt[:, :, :], dg(27 + j),
                        VD52[:, 1 + 4 * c: 1 + 4 * c + 4, j: j + 96],
                        start=(j == 0), stop=(j == 2))
                nc.vector.tensor_copy(out=RD[:, 4 * c: 4 * c + 4, :], in_=pt[:, :, :])

            # res = conv3x3(VD, e9); NUM = res - rd (fused eviction)
            for c in range(12):
                pt = psum.tile([128, 4, 96], F32, tag="pB")
                for k in range(9):
                    di, dj = divmod(k, 3)
                    nc.tensor.matmul(
                        pt[:, :, :], dg(18 + k),
                        VD52[:, di + 4 * c: di + 4 * c + 4, dj: dj + 96],
                        start=(k == 0), stop=(k == 8))
                nc.vector.tensor_sub(NUM[:, 4 * c: 4 * c + 4, :], pt[:, :, :],
                                     RD[:, 4 * c: 4 * c + 4, :])

            # ---- y1: 9-neighbor sigmoid diffusion (fp16 chain) ----
            vdc = VD16[:, 1:49, 1:97]
            first = True
            for k in range(9):
                di, dj = divmod(k, 3)
                sv = V16[:, di + 1: di + 49, dj: dj + 96]
                if k == 4:
                    # d == 0 -> sigmoid == 0.5, folded into tab (0.5 * w9_4)
                    nc.vector.scalar_tensor_tensor(
                        out=Y1[:, :, :], in0=sv, scalar=TAB[:, 4:5], in1=Y1[:, :, :],
                        op0=AluOpType.mult, op1=AluOpType.add)
                    continue
                svd = VD16[:, di: di + 48, dj: dj + 96]
                td = pool.tile([128, 48, 96], F16, tag="TD", bufs=3)
                nc.vector.tensor_sub(td[:, :, :], svd, vdc)
                # |x| = max(-x, x) in one fused op
                nc.vector.scalar_tensor_tensor(
                    out=td[:, :, :], in0=td[:, :, :], scalar=-1.0, in1=td[:, :, :],
                    op0=AluOpType.mult, op1=AluOpType.max)
                nc.scalar.activation(td[:, :, :], td[:, :, :], AF.Sigmoid)
                if first:
                    nc.vector.scalar_tensor_tensor(
                        out=Y1[:, :, :], in0=td[:, :, :], scalar=TAB[:, k:k + 1],
                        in1=sv, op0=AluOpType.mult, op1=AluOpType.mult)
                    first = False
                else:
                    nc.vector.scalar_tensor_tensor(
                        out=td[:, :, :], in0=td[:, :, :], scalar=TAB[:, k:k + 1],
                        in1=sv, op0=AluOpType.mult, op1=AluOpType.mult)
                    nc.vector.tensor_add(Y1[:, :, :], Y1[:, :, :], td[:, :, :])

            # ---- finale ----
            nc.vector.tensor_mul(Y2[:, :, :], NUM[:, :, :], RCP[:, :, :])
            # yt_int (rows plane 0..47 -> YT rows 1..49): o1 = 2*yt - y1 in place
            nc.vector.scalar_tensor_tensor(
                out=YT[:, 1:49, :], in0=YT[:, 1:49, :], scalar=2.0, in1=Y1[:, :, :],
                op0=AluOpType.mult, op1=AluOpType.subtract)
            nc.vector.tensor_sub(YO[:, :, :], YT[:, 1:49, :], Y2[:, :, :])
            nc.sync.dma_start(out=y_out[:, 0:HH, :], in_=YO[0:64, :, :])
            nc.sync.dma_start(out=y_out[:, HH:H, :], in_=YO[64:128, :, :])
    return nc


def _get_nc(name):
    if name not in _BUILD_CACHE:
        _BUILD_CACHE[name] = _build_gap_nc() if name == "gap" else _build_main_nc()
    return _BUILD_CACHE[name]


# --------------------------------------------------------------------------- #
# entry point
# --------------------------------------------------------------------------- #

def kernel(query, value, hard_sigmoid_masks, w_dp, b_dp, bn_g, bn_b, bn_rm, bn_rv,
           w_dc, b_dc, w_lin, b_lin, w_delin, b_delin):
    inputs = dict(query=query, value=value, w_dp=w_dp, b_dp=b_dp, bn_g=bn_g,
                  bn_b=bn_b, bn_rm=bn_rm, bn_rv=bn_rv, w_dc=w_dc, b_dc=b_dc,
                  w_lin=w_lin, b_lin=b_lin, w_delin=w_delin, b_delin=b_delin)
    value = np.ascontiguousarray(np.asarray(value, np.float32))
    planes = value.reshape(NPLANES, H, W)
    slices = [np.ascontiguousarray(planes[i * PPC:(i + 1) * PPC])
              for i in range(N_CORES)]
    core_ids = list(range(N_CORES))

    def _run(nc, in_maps):
        # the axon terminal occasionally wedges transiently after an earlier
        # fault; retry with backoff
        import time
        last = None
        for attempt in range(4):
            try:
                return run_bass_kernel_spmd(nc, in_maps, core_ids)
            except Exception as e:  # noqa: BLE001
                last = e
                time.sleep(20 * (attempt + 1))
        raise last

    # launch A: per-plane sums
    res = _run(_get_nc("gap"), [{"xv": s} for s in slices])
    gsum = np.concatenate([r["gsum"][:PPC, 0] + r["gsum"][PPC:, 0]
                           for r in res.results])          # (512,)
    gap = (gsum / (H * W)).reshape(B, C)

    # host: filter scalars + q head
    f9, w9, e9, qout = _gate_chain_host(inputs, gap)
    ae9 = np.abs(e9)
    wd = e9.reshape(NPLANES, 3, 3).sum(axis=1)             # (512,3) col sums
    w9h = w9.copy()
    w9h[:, 4] *= 0.5

    in_maps = []
    for i in range(N_CORES):
        sl = slice(i * PPC, (i + 1) * PPC)
        coef = np.concatenate([f9[sl], ae9[sl], e9[sl], wd[sl]], axis=1)  # (64,30)
        dg = np.zeros((N_DIAG, 128, 128), np.float32)
        idx = np.arange(128)
        dg[:, idx, idx] = coef.T[:, idx % PPC]
        tab = np.zeros((128, TAB_COLS), np.float32)
        tab[:, 0:9] = w9h[sl][idx % PPC]
        in_maps.append({"xv": slices[i], "diags": dg, "tab": tab})

    res = _run(_get_nc("main"), in_maps)
    y = np.concatenate([r["y"] for r in res.results]).reshape(B, C, H, W)
    return qout, y


if __name__ == "__main__":
    rng = np.random.default_rng(0)
    pass


# revision 13
# speedup vs baseline: 1.2945x; 1.2945x over previous
"""Trainium2 Bass kernel for nn_GCBlock9 (dynamic depthwise smoothing + 9-neighbor
sigmoid diffusion block).

Contract: kernel(**inputs) takes the FULL unsharded inputs (as from
setup_inputs()) and returns the full output tuple (q, y) matching the
reference. Self-contained: shapes/sharding hardcoded.

Distribution: the 512 (b, c) planes are data-parallel; 64 planes per core on 8
cores. On-core layout: 128 partitions = (half, plane) with each partition
holding a 48-row half-plane in padded buffers so every conv tap / 9-neighbor
shift is a pure free-dim AP offset (halo rows recomputed redundantly).

The tiny dense chain (GAP -> 1x1 convs -> 27 per-plane filter scalars, and the
q output head) couples all 256 channels, so it is split off: launch A reduces
each plane on-device, the filter scalars are formed on host (f64), and launch B
does the heavy per-plane work:
  - TensorE: all 4 depthwise convs as diagonal-matmul PSUM accumulations
    (per-plane weights on the diagonal, shifts as rhs AP offsets), fp32.
  - VectorE: fused scalar_tensor_tensor MACs, the fp16 sigmoid-diffusion chain,
    reciprocal, final combine.
  - ScalarE: Square/Exp/Sigmoid LUTs + PSUM evictions.
"""

import json

import numpy as np

import concourse.bass as bass
import concourse.mybir as mybir
from concourse.alu_op_type import AluOpType
from concourse.bass_utils import run_bass_kernel_spmd
from concourse.tile import TileContext

# The walrus build in this container accepts at most one sync-wait command per
# instruction; Tile emits several (one per producer semaphore). Hoist the
# excess onto standalone EventSemaphore waits (same engine, immediately before
# the consumer — exactly what raw-bass wait_ge() emits), via a to_json_bytes
# post-pass so both compile paths (direct + bass2jax/PJRT) are covered.
_MAX_WAITS = 1


def _split_sync_waits(js: bytes) -> bytes:
    data = json.loads(js)
    n = 0
    for f in data.get("functions", []):
        for bb in f.get("blocks", []):
            out = []
            for ins in bb.get("instructions", []):
                si = ins.get("sync_info")
                if si:
                    w = si.get("on_wait") or []
                    if len(w) > _MAX_WAITS:
                        excess, keep = w[:-_MAX_WAITS], w[-_MAX_WAITS:]
                        for x in excess:
                            n += 1
                            out.append({
                                "debug": ins.get("debug"),
                                "engine": ins["engine"],
                                "ins": [], "outs": [],
                                "name": f"{ins['name']}_hw{n}",
                                "opcode": "EventSemaphore",
                                "sync_info": {"on_update": [], "on_wait": [x]}})
                        si["on_wait"] = keep
                out.append(ins)
            bb["instructions"] = out
    return json.dumps(data).encode()


if not getattr(bass.Bass, "_wait_split_installed", False):
    _orig_to_json = bass.Bass.to_json_bytes
    bass.Bass.to_json_bytes = (
        lambda self, *a, **k: _split_sync_waits(_orig_to_json(self, *a, **k)))
    bass.Bass._wait_split_installed = True

F32 = mybir.dt.float32
F16 = mybir.dt.float16
AF = mybir.ActivationFunctionType

N_CORES = 8
B, C, H, W = 2, 256, 96, 96
NPLANES = B * C                 # 512
PPC = NPLANES // N_CORES        # 64 planes per core
HH = H // 2                     # 48 rows per half-plane

N_DIAG = 30                     # 9 f9 + 9 |e9| + 9 e9 + 3 wd
TAB_COLS = 16                   # w9 (center pre-halved) in cols 0..8

_BUILD_CACHE = {}


# --------------------------------------------------------------------------- #
# host-side dense chain (0.1% of FLOPs; couples all channels)
# --------------------------------------------------------------------------- #

def _gate_chain_host(inputs, gap):
    dt = np.float64
    q = np.asarray(inputs['query'], dt)                      # (100,B,C)
    w_dp = np.asarray(inputs['w_dp'], dt); b_dp = np.asarray(inputs['b_dp'], dt)
    bn_g = np.asarray(inputs['bn_g'], dt); bn_b = np.asarray(inputs['bn_b'], dt)
    bn_rm = np.asarray(inputs['bn_rm'], dt); bn_rv = np.asarray(inputs['bn_rv'], dt)
    w_dc = np.asarray(inputs['w_dc'], dt); b_dc = np.asarray(inputs['b_dc'], dt)
    w_lin = np.asarray(inputs['w_lin'], dt); b_lin = np.asarray(inputs['b_lin'], dt)
    w_delin = np.asarray(inputs['w_delin'], dt); b_delin = np.asarray(inputs['b_delin'], dt)

    x = gap.astype(dt) @ w_dp.T + b_dp                       # (B,4C)
    x = (x - bn_rm) / np.sqrt(bn_rv + 1e-5) * bn_g + bn_b
    x = np.maximum(x, 0)
    dyn = 1.0 / (1.0 + np.exp(-(x @ w_dc.T + b_dc)))         # (B,3C)
    d_f, d_w, d_e = np.split(dyn, 3, axis=-1)                # (B,C)
    lin = np.einsum('qbc,kq->bck', q, w_lin) + b_lin         # (B,C,27)
    f9 = lin[..., 0:9] * d_f[..., None]
    w9 = lin[..., 9:18] * d_w[..., None]
    e9 = lin[..., 18:27] * d_e[..., None]
    qcat = np.concatenate([f9, w9, e9], axis=-1)             # (B,C,27)
    qout = (qcat @ w_delin.T + b_delin).transpose(2, 0, 1)   # (100,B,C)
    return (f9.reshape(NPLANES, 9).astype(np.float32),
            w9.reshape(NPLANES, 9).astype(np.float32),
            e9.reshape(NPLANES, 9).astype(np.float32),
            qout.astype(np.float32))


# --------------------------------------------------------------------------- #
# launch A: per-plane sums (for GAP)
# --------------------------------------------------------------------------- #

def _build_gap_nc():
    nc = bass.Bass()
    xv = nc.dram_tensor("xv", [PPC, H, W], F32, kind="ExternalInput")
    gs = nc.dram_tensor("gsum", [128, 1], F32, kind="ExternalOutput")
    with TileContext(nc) as tc:
        with tc.tile_pool(name="p", bufs=1) as pool:
            g = pool.tile([128, HH * W], F32)
            s = pool.tile([128, 1], F32)
            nc.sync.dma_start(out=g[0:PPC, :], in_=xv[:, 0:HH, :])
            nc.sync.dma_start(out=g[PPC:128, :], in_=xv[:, HH:H, :])
            nc.vector.tensor_reduce(
                out=s[0:PPC, :], in_=g[0:PPC, :], axis=mybir.AxisListType.X,
                op=AluOpType.add)
            nc.vector.tensor_reduce(
                out=s[PPC:128, :], in_=g[PPC:128, :], axis=mybir.AxisListType.X,
                op=AluOpType.add)
            nc.sync.dma_start(out=gs[:, :], in_=s[:, :])
    return nc


# --------------------------------------------------------------------------- #
# launch B: the per-plane heavy math
# --------------------------------------------------------------------------- #

def _build_main_nc():
    nc = bass.Bass()
    xv = nc.dram_tensor("xv", [PPC, H, W], F32, kind="ExternalInput")
    diags = nc.dram_tensor("diags", [N_DIAG, 128, 128], F16, kind="ExternalInput")
    tabd = nc.dram_tensor("tab", [128, TAB_COLS], F32, kind="ExternalInput")
    y_out = nc.dram_tensor("y", [PPC, H, W], F32, kind="ExternalOutput")

    with TileContext(nc) as tc:
        with tc.tile_pool(name="sb", bufs=1) as pool, \
             tc.tile_pool(name="ps", bufs=4, space="PSUM") as psum:

            V52 = pool.tile([128, 52, 98], F32)       # padded value, 2-row halo
            V16 = pool.tile([128, 52, 98], F16)
            YT = pool.tile([128, 50, 96], F32)        # ytemp (rows: plane r-1)
            D52 = pool.tile([128, 50, 98], F16)       # diff, padded, 1-row halo
            VD52 = pool.tile([128, 50, 98], F16)      # value*diff, padded
            DEN = pool.tile([128, 48, 96], F32, tag="V52")   # reuse V52 memory
            RCP = pool.tile([128, 48, 96], F32, tag="D52")   # reuse D52 memory
            RD = pool.tile([128, 48, 96], F32)
            NUM = pool.tile([128, 48, 96], F32)
            Y2 = pool.tile([128, 48, 96], F32, tag="RD")     # reuse RD memory
            Y1 = pool.tile([128, 48, 96], F16)
            YO = pool.tile([128, 48, 96], F32)
            TAB = pool.tile([128, TAB_COLS], F32)
            DG = pool.tile([128, N_DIAG, 128], F16)
            EPS = pool.tile([128, 1], F32)
            nc.vector.memset(EPS[:, :], 1e-10)

            # ---- loads + padding zeros ----
            nc.sync.dma_start(out=TAB[:, :], in_=tabd[:, :])
            nc.sync.dma_start(out=DG[:, :, :], in_=diags.rearrange("d p m -> p d m"))
            nc.vector.memset(V52[0:64, 0:2, :], 0.0)       # plane rows -2,-1
            nc.vector.memset(V52[64:128, 50:52, :], 0.0)   # plane rows 96,97
            nc.vector.memset(V52[:, :, 0:1], 0.0)
            nc.vector.memset(V52[:, :, 97:98], 0.0)
            nc.vector.memset(D52[:, :, 0:1], 0.0)
            nc.vector.memset(D52[:, :, 97:98], 0.0)
            nc.vector.memset(VD52[:, :, 0:1], 0.0)
            nc.vector.memset(VD52[:, :, 97:98], 0.0)
            # half0 covers plane rows -2..49, half1 covers 46..97
            nc.sync.dma_start(out=V52[0:64, 2:52, 1:97], in_=xv[:, 0:50, :])
            nc.sync.dma_start(out=V52[64:128, 0:50, 1:97], in_=xv[:, 46:96, :])

            def dg(i):
                return DG[:, i, :]

            nc.vector.tensor_copy(out=V16[:, :, :], in_=V52[:, :, :])

            # ---- phase A: ytemp = conv3x3(V, f9) on TensorE (10 x 5-row chunks)
            for c in range(10):
                pt = psum.tile([128, 5, 96], F32, tag="pA")
                for k in range(9):
                    di, dj = divmod(k, 3)
                    nc.tensor.matmul(
                        pt[:, :, :], dg(k),
                        V16[:, di + 5 * c: di + 5 * c + 5, dj: dj + 96],
                        start=(k == 0), stop=(k == 8))
                nc.scalar.copy(YT[:, 5 * c: 5 * c + 5, :], pt[:, :, :])

            # D = exp(-(V - ytemp)^2) ; VD = V * D   (rows = plane r-1, 50 rows)
            nc.vector.tensor_sub(D52[:, :, 1:97], V16[:, 1:51, 1:97], YT[:, :, :])
            nc.scalar.activation(D52[:, :, 1:97], D52[:, :, 1:97], AF.Square)
            nc.scalar.activation(D52[:, :, 1:97], D52[:, :, 1:97], AF.Exp, scale=-1.0)
            # plane-boundary halo rows of D must be zero (zero-pad semantics)
            nc.vector.memset(D52[0:64, 0:1, 1:97], 0.0)
            nc.vector.memset(D52[64:128, 49:50, 1:97], 0.0)
            nc.vector.tensor_mul(VD52[:, :, 1:97], V16[:, 1:51, 1:97], D52[:, :, 1:97])

            # ---- phase B convs on TensorE (12 x 4-row chunks each) ----
            # yd9 = conv3x3(D, |e9|) -> DEN = yd9 + 1e-10 (fused in eviction)
            for c in range(12):
                pt = psum.tile([128, 4, 96], F32, tag="pB")
                for k in range(9):
                    di, dj = divmod(k, 3)
                    nc.tensor.matmul(
                        pt[:, :, :], dg(9 + k),
                        D52[:, di + 4 * c: di + 4 * c + 4, dj: dj + 96],
                        start=(k == 0), stop=(k == 8))
                nc.scalar.activation(DEN[:, 4 * c: 4 * c + 4, :], pt[:, :, :],
                                     AF.Identity, bias=EPS[:, 0:1])
            # 1/den as exp(-ln(den)): both LUTs live in the same ACT table set
            # as Square/Exp (natural_log_exp_and_others), ~1e-5 rel err.
            nc.scalar.activation(RCP[:, :, :], DEN[:, :, :], AF.Ln)
            nc.scalar.activation(RCP[:, :, :], RCP[:, :, :], AF.Exp, scale=-1.0)

            # rd = conv1x3(VD, wd)
            for c in range(12):
                pt = psum.tile([128, 4, 96], F32, tag="pB")
                for j in range(3):
                    nc.tensor.matmul(
                        pt[:, :, :], dg(27 + j),
                        VD52[:, 1 + 4 * c: 1 + 4 * c + 4, j: j + 96],
                        start=(j == 0), stop=(j == 2))
                nc.vector.tensor_copy(out=RD[:, 4 * c: 4 * c + 4, :], in_=pt[:, :, :])

            # res = conv3x3(VD, e9); NUM = res - rd (fused eviction)
            for c in range(12):
                pt = psum.tile([128, 4, 96], F32, tag="pB")
                for k in range(9):
                    di, dj = divmod(k, 3)
                    nc.tensor.matmul(
                        pt[:, :, :], dg(18 + k),
                        VD52[:, di + 4 * c: di + 4 * c + 4, dj: dj + 96],
                        start=(k == 0), stop=(k == 8))
                nc.vector.tensor_sub(NUM[:, 4 * c: 4 * c + 4, :], pt[:, :, :],
                                     RD[:, 4 * c: 4 * c + 4, :])

            # ---- y1: 9-neighbor sigmoid diffusion (fp16 chain) ----
            vdc = VD52[:, 1:49, 1:97]
            first = True
            for k in range(9):
                di, dj = divmod(k, 3)
                sv = V16[:, di + 1: di + 49, dj: dj + 96]
                if k == 4:
                    # d == 0 -> sigmoid == 0.5, folded into tab (0.5 * w9_4)
                    nc.vector.scalar_tensor_tensor(
                        out=Y1[:, :, :], in0=sv, scalar=TAB[:, 4:5], in1=Y1[:, :, :],
                        op0=AluOpType.mult, op1=AluOpType.add)
                    continue
                svd = VD52[:, di: di + 48, dj: dj + 96]
                td = pool.tile([128, 48, 96], F16, tag="TD", bufs=3)
                nc.vector.tensor_sub(td[:, :, :], svd, vdc)
                # |x| = max(-x, x) in one fused op
                nc.vector.scalar_tensor_tensor(
                    out=td[:, :, :], in0=td[:, :, :], scalar=-1.0, in1=td[:, :, :],
                    op0=AluOpType.mult, op1=AluOpType.max)
                nc.scalar.activation(td[:, :, :], td[:, :, :], AF.Sigmoid)
                if first:
                    nc.vector.scalar_tensor_tensor(
                        out=Y1[:, :, :], in0=td[:, :, :], scalar=TAB[:, k:k + 1],
                        in1=sv, op0=AluOpType.mult, op1=AluOpType.mult)
                    first = False
                else:
                    nc.vector.scalar_tensor_tensor(
                        out=td[:, :, :], in0=td[:, :, :], scalar=TAB[:, k:k + 1],
                        in1=sv, op0=AluOpType.mult, op1=AluOpType.mult)
                    nc.vector.tensor_add(Y1[:, :, :], Y1[:, :, :], td[:, :, :])

            # ---- finale ----
            nc.vector.tensor_mul(Y2[:, :, :], NUM[:, :, :], RCP[:, :, :])
            # yt_int (rows plane 0..47 -> YT rows 1..49): o1 = 2*yt - y1 in place
            nc.vector.scalar_tensor_tensor(
                out=YT[:, 1:49, :], in0=YT[:, 1:49, :], scalar=2.0, in1=Y1[:, :, :],
                op0=AluOpType.mult, op1=AluOpType.subtract)
            nc.vector.tensor_sub(YO[:, :, :], YT[:, 1:49, :], Y2[:, :, :])
            nc.sync.dma_start(out=y_out[:, 0:HH, :], in_=YO[0:64, :, :])
            nc.sync.dma_start(out=y_out[:, HH:H, :], in_=YO[64:128, :, :])
    return nc


def _get_nc(name):
    if name not in _BUILD_CACHE:
        _BUILD_CACHE[name] = _build_gap_nc() if name == "gap" else _build_main_nc()
    return _BUILD_CACHE[name]


# --------------------------------------------------------------------------- #
# entry point
# --------------------------------------------------------------------------- #

def kernel(query, value, hard_sigmoid_masks, w_dp, b_dp, bn_g, bn_b, bn_rm, bn_rv,
           w_dc, b_dc, w_lin, b_lin, w_delin, b_delin):
    inputs = dict(query=query, value=value, w_dp=w_dp, b_dp=b_dp, bn_g=bn_g,
                  bn_b=bn_b, bn_rm=bn_rm, bn_rv=bn_rv, w_dc=w_dc, b_dc=b_dc,
                  w_lin=w_lin, b_lin=b_lin, w_delin=w_delin, b_delin=b_delin)
    value = np.ascontiguousarray(np.asarray(value, np.float32))
    planes = value.reshape(NPLANES, H, W)
    slices = [np.ascontiguousarray(planes[i * PPC:(i + 1) * PPC])
              for i in range(N_CORES)]
    core_ids = list(range(N_CORES))

    def _run(nc, in_maps):
        # the axon terminal occasionally wedges transiently after an earlier
        # fault; retry with backoff
        import time
        last = None
        for attempt in range(4):
            try:
                return run_bass_kernel_spmd(nc, in_maps, core_ids)
            except Exception as e:  # noqa: BLE001
                last = e
                time.sleep(20 * (attempt + 1))
        raise last

    # launch A: per-plane sums
    res = _run(_get_nc("gap"), [{"xv": s} for s in slices])
    gsum = np.concatenate([r["gsum"][:PPC, 0] + r["gsum"][PPC:, 0]
                           for r in res.results])          # (512,)
    gap = (gsum / (H * W)).reshape(B, C)

    # host: filter scalars + q head
    f9, w9, e9, qout = _gate_chain_host(inputs, gap)
    ae9 = np.abs(e9)
    wd = e9.reshape(NPLANES, 3, 3).sum(axis=1)             # (512,3) col sums
    w9h = w9.copy()
    w9h[:, 4] *= 0.5

    in_maps = []
    for i in range(N_CORES):
        sl = slice(i * PPC, (i + 1) * PPC)
        coef = np.concatenate([f9[sl], ae9[sl], e9[sl], wd[sl]], axis=1)  # (64,30)
        dg = np.zeros((N_DIAG, 128, 128), np.float16)
        idx = np.arange(128)
        dg[:, idx, idx] = coef.T[:, idx % PPC]
        tab = np.zeros((128, TAB_COLS), np.float32)
        tab[:, 0:9] = w9h[sl][idx % PPC]
        in_maps.append({"xv": slices[i], "diags": dg, "tab": tab})

    res = _run(_get_nc("main"), in_maps)
    y = np.concatenate([r["y"] for r in res.results]).reshape(B, C, H, W)
    return qout, y


if __name__ == "__main__":
    rng = np.random.default_rng(0)
    pass


# revision 16
# speedup vs baseline: 1.3751x; 1.0623x over previous
"""Trainium2 Bass kernel for nn_GCBlock9 (dynamic depthwise smoothing + 9-neighbor
sigmoid diffusion block).

Contract: kernel(**inputs) takes the FULL unsharded inputs (as from
setup_inputs()) and returns the full output tuple (q, y) matching the
reference. Self-contained: shapes/sharding hardcoded.

Distribution: the 512 (b, c) planes are data-parallel; 64 planes per core on 8
cores. On-core layout: 128 partitions = (half, plane) with each partition
holding a 48-row half-plane in padded buffers so every conv tap / 9-neighbor
shift is a pure free-dim AP offset (halo rows recomputed redundantly).

The tiny dense chain (GAP -> 1x1 convs -> 27 per-plane filter scalars, and the
q output head) couples all 256 channels, so it is split off: launch A reduces
each plane on-device, the filter scalars are formed on host (f64), and launch B
does the heavy per-plane work:
  - TensorE: all 4 depthwise convs as diagonal-matmul PSUM accumulations
    (per-plane weights on the diagonal, shifts as rhs AP offsets), fp32.
  - VectorE: fused scalar_tensor_tensor MACs, the fp16 sigmoid-diffusion chain,
    reciprocal, final combine.
  - ScalarE: Square/Exp/Sigmoid LUTs + PSUM evictions.
"""

import json

import numpy as np

import concourse.bass as bass
import concourse.mybir as mybir
from concourse.alu_op_type import AluOpType
from concourse.bass_utils import run_bass_kernel_spmd
from concourse.tile import TileContext

# The walrus build in this container accepts at most one sync-wait command per
# instruction; Tile emits several (one per producer semaphore). Hoist the
# excess onto standalone EventSemaphore waits (same engine, immediately before
# the consumer — exactly what raw-bass wait_ge() emits), via a to_json_bytes
# post-pass so both compile paths (direct + bass2jax/PJRT) are covered.
_MAX_WAITS = 1


def _split_sync_waits(js: bytes) -> bytes:
    data = json.loads(js)
    n = 0
    for f in data.get("functions", []):
        for bb in f.get("blocks", []):
            out = []
            for ins in bb.get("instructions", []):
                si = ins.get("sync_info")
                if si:
                    w = si.get("on_wait") or []
                    if len(w) > _MAX_WAITS:
                        excess, keep = w[:-_MAX_WAITS], w[-_MAX_WAITS:]
                        for x in excess:
                            n += 1
                            out.append({
                                "debug": ins.get("debug"),
                                "engine": ins["engine"],
                                "ins": [], "outs": [],
                                "name": f"{ins['name']}_hw{n}",
                                "opcode": "EventSemaphore",
                                "sync_info": {"on_update": [], "on_wait": [x]}})
                        si["on_wait"] = keep
                out.append(ins)
            bb["instructions"] = out
    return json.dumps(data).encode()


if not getattr(bass.Bass, "_wait_split_installed", False):
    _orig_to_json = bass.Bass.to_json_bytes
    bass.Bass.to_json_bytes = (
        lambda self, *a, **k: _split_sync_waits(_orig_to_json(self, *a, **k)))
    bass.Bass._wait_split_installed = True

F32 = mybir.dt.float32
F16 = mybir.dt.float16
AF = mybir.ActivationFunctionType

N_CORES = 8
B, C, H, W = 2, 256, 96, 96
NPLANES = B * C                 # 512
PPC = NPLANES // N_CORES        # 64 planes per core
HH = H // 2                     # 48 rows per half-plane

N_DIAG = 30                     # 9 f9 + 9 |e9| + 9 e9 + 3 wd
TAB_COLS = 16                   # w9 (center pre-halved) in cols 0..8

_BUILD_CACHE = {}


# --------------------------------------------------------------------------- #
# host-side dense chain (0.1% of FLOPs; couples all channels)
# --------------------------------------------------------------------------- #

def _gate_chain_host(inputs, gap):
    dt = np.float64
    q = np.asarray(inputs['query'], dt)                      # (100,B,C)
    w_dp = np.asarray(inputs['w_dp'], dt); b_dp = np.asarray(inputs['b_dp'], dt)
    bn_g = np.asarray(inputs['bn_g'], dt); bn_b = np.asarray(inputs['bn_b'], dt)
    bn_rm = np.asarray(inputs['bn_rm'], dt); bn_rv = np.asarray(inputs['bn_rv'], dt)
    w_dc = np.asarray(inputs['w_dc'], dt); b_dc = np.asarray(inputs['b_dc'], dt)
    w_lin = np.asarray(inputs['w_lin'], dt); b_lin = np.asarray(inputs['b_lin'], dt)
    w_delin = np.asarray(inputs['w_delin'], dt); b_delin = np.asarray(inputs['b_delin'], dt)

    x = gap.astype(dt) @ w_dp.T + b_dp                       # (B,4C)
    x = (x - bn_rm) / np.sqrt(bn_rv + 1e-5) * bn_g + bn_b
    x = np.maximum(x, 0)
    dyn = 1.0 / (1.0 + np.exp(-(x @ w_dc.T + b_dc)))         # (B,3C)
    d_f, d_w, d_e = np.split(dyn, 3, axis=-1)                # (B,C)
    lin = np.einsum('qbc,kq->bck', q, w_lin) + b_lin         # (B,C,27)
    f9 = lin[..., 0:9] * d_f[..., None]
    w9 = lin[..., 9:18] * d_w[..., None]
    e9 = lin[..., 18:27] * d_e[..., None]
    qcat = np.concatenate([f9, w9, e9], axis=-1)             # (B,C,27)
    qout = (qcat @ w_delin.T + b_delin).transpose(2, 0, 1)   # (100,B,C)
    return (f9.reshape(NPLANES, 9).astype(np.float32),
            w9.reshape(NPLANES, 9).astype(np.float32),
            e9.reshape(NPLANES, 9).astype(np.float32),
            qout.astype(np.float32))


# --------------------------------------------------------------------------- #
# launch A: per-plane sums (for GAP)
# --------------------------------------------------------------------------- #

def _build_gap_nc():
    nc = bass.Bass()
    xv = nc.dram_tensor("xv", [PPC, H, W], F32, kind="ExternalInput")
    gs = nc.dram_tensor("gsum", [128, 1], F32, kind="ExternalOutput")
    with TileContext(nc) as tc:
        with tc.tile_pool(name="p", bufs=1) as pool:
            g = pool.tile([128, HH * W], F32)
            s = pool.tile([128, 1], F32)
            nc.sync.dma_start(out=g[0:PPC, :], in_=xv[:, 0:HH, :])
            nc.sync.dma_start(out=g[PPC:128, :], in_=xv[:, HH:H, :])
            nc.vector.tensor_reduce(
                out=s[0:PPC, :], in_=g[0:PPC, :], axis=mybir.AxisListType.X,
                op=AluOpType.add)
            nc.vector.tensor_reduce(
                out=s[PPC:128, :], in_=g[PPC:128, :], axis=mybir.AxisListType.X,
                op=AluOpType.add)
            nc.sync.dma_start(out=gs[:, :], in_=s[:, :])
    return nc


# --------------------------------------------------------------------------- #
# launch B: the per-plane heavy math
# --------------------------------------------------------------------------- #

def _build_main_nc():
    nc = bass.Bass()
    xv = nc.dram_tensor("xv", [PPC, H, W], F32, kind="ExternalInput")
    diags = nc.dram_tensor("diags", [N_DIAG, 128, 128], F16, kind="ExternalInput")
    tabd = nc.dram_tensor("tab", [128, TAB_COLS], F32, kind="ExternalInput")
    y_out = nc.dram_tensor("y", [PPC, H, W], F32, kind="ExternalOutput")

    with TileContext(nc) as tc:
        with tc.tile_pool(name="sb", bufs=1) as pool, \
             tc.tile_pool(name="ps", bufs=4, space="PSUM") as psum:

            V52 = pool.tile([128, 52, 98], F32)       # padded value, 2-row halo
            V16 = pool.tile([128, 52, 98], F16)
            YT = pool.tile([128, 50, 96], F32)        # ytemp (rows: plane r-1)
            D52 = pool.tile([128, 50, 98], F16)       # diff, padded, 1-row halo
            VD52 = pool.tile([128, 50, 98], F16)      # value*diff, padded
            DEN = pool.tile([128, 48, 96], F32, tag="V52")   # reuse V52 memory
            RCP = pool.tile([128, 48, 96], F32, tag="D52")   # reuse D52 memory
            RD = pool.tile([128, 48, 96], F32)
            NUM = pool.tile([128, 48, 96], F32)
            Y2 = pool.tile([128, 48, 96], F32, tag="RD")     # reuse RD memory
            Y1 = pool.tile([128, 48, 96], F16)
            YO = pool.tile([128, 48, 96], F32)
            TAB = pool.tile([128, TAB_COLS], F32)
            DG = pool.tile([128, N_DIAG, 128], F16)
            EPS = pool.tile([128, 1], F32)
            nc.vector.memset(EPS[:, :], 1e-10)

            # ---- loads + padding zeros ----
            nc.sync.dma_start(out=TAB[:, :], in_=tabd[:, :])
            nc.sync.dma_start(out=DG[:, :, :], in_=diags.rearrange("d p m -> p d m"))
            nc.vector.memset(V52[0:64, 0:2, :], 0.0)       # plane rows -2,-1
            nc.vector.memset(V52[64:128, 50:52, :], 0.0)   # plane rows 96,97
            nc.vector.memset(V52[:, :, 0:1], 0.0)
            nc.vector.memset(V52[:, :, 97:98], 0.0)
            nc.vector.memset(D52[:, :, 0:1], 0.0)
            nc.vector.memset(D52[:, :, 97:98], 0.0)
            nc.vector.memset(VD52[:, :, 0:1], 0.0)
            nc.vector.memset(VD52[:, :, 97:98], 0.0)
            # half0 covers plane rows -2..49, half1 covers 46..97
            nc.sync.dma_start(out=V52[0:64, 2:52, 1:97], in_=xv[:, 0:50, :])
            nc.sync.dma_start(out=V52[64:128, 0:50, 1:97], in_=xv[:, 46:96, :])

            def dg(i):
                return DG[:, i, :]

            nc.vector.tensor_copy(out=V16[:, :, :], in_=V52[:, :, :])

            # ---- phase A: ytemp = conv3x3(V, f9) on TensorE (10 x 5-row chunks)
            for h in range(2):
                pts = [psum.tile([128, 5, 96], F32, name=f"pa{c}", tag=f"pB{c}", bufs=1)
                       for c in range(5)]
                for k in range(9):
                    di, dj = divmod(k, 3)
                    for c in range(5):
                        r = 5 * (5 * h + c)
                        nc.tensor.matmul(
                            pts[c][:, :, :], dg(k),
                            V16[:, di + r: di + r + 5, dj: dj + 96],
                            start=(k == 0), stop=(k == 8))
                for c in range(5):
                    r = 5 * (5 * h + c)
                    nc.scalar.copy(YT[:, r: r + 5, :], pts[c][:, :, :])

            # D = exp(-(V - ytemp)^2) ; VD = V * D   (rows = plane r-1, 50 rows)
            nc.vector.tensor_sub(D52[:, :, 1:97], V16[:, 1:51, 1:97], YT[:, :, :])
            nc.scalar.activation(D52[:, :, 1:97], D52[:, :, 1:97], AF.Square)
            nc.scalar.activation(D52[:, :, 1:97], D52[:, :, 1:97], AF.Exp, scale=-1.0)
            # plane-boundary halo rows of D must be zero (zero-pad semantics)
            nc.vector.memset(D52[0:64, 0:1, 1:97], 0.0)
            nc.vector.memset(D52[64:128, 49:50, 1:97], 0.0)
            nc.vector.tensor_mul(VD52[:, :, 1:97], V16[:, 1:51, 1:97], D52[:, :, 1:97])

            # ---- phase B convs on TensorE (12 x 4-row chunks each) ----
            # yd9 = conv3x3(D, |e9|) -> DEN = yd9 + 1e-10 (fused in eviction)
            for h in range(2):
                pts = [psum.tile([128, 4, 96], F32, name=f"pb{c}", tag=f"pB{c}", bufs=1)
                       for c in range(6)]
                for k in range(9):
                    di, dj = divmod(k, 3)
                    for c in range(6):
                        r = 4 * (6 * h + c)
                        nc.tensor.matmul(
                            pts[c][:, :, :], dg(9 + k),
                            D52[:, di + r: di + r + 4, dj: dj + 96],
                            start=(k == 0), stop=(k == 8))
                for c in range(6):
                    r = 4 * (6 * h + c)
                    nc.scalar.activation(DEN[:, r: r + 4, :], pts[c][:, :, :],
                                         AF.Identity, bias=EPS[:, 0:1])
            # 1/den as exp(-ln(den)): both LUTs live in the same ACT table set
            # as Square/Exp (natural_log_exp_and_others), ~1e-5 rel err.
            nc.scalar.activation(RCP[:, :, :], DEN[:, :, :], AF.Ln)
            nc.scalar.activation(RCP[:, :, :], RCP[:, :, :], AF.Exp, scale=-1.0)

            # rd = conv1x3(VD, wd)
            for h in range(2):
                pts = [psum.tile([128, 4, 96], F32, name=f"pb{c}", tag=f"pB{c}", bufs=1)
                       for c in range(6)]
                for j in range(3):
                    for c in range(6):
                        r = 4 * (6 * h + c)
                        nc.tensor.matmul(
                            pts[c][:, :, :], dg(27 + j),
                            VD52[:, 1 + r: 1 + r + 4, j: j + 96],
                            start=(j == 0), stop=(j == 2))
                for c in range(6):
                    r = 4 * (6 * h + c)
                    nc.vector.tensor_copy(out=RD[:, r: r + 4, :], in_=pts[c][:, :, :])

            # res = conv3x3(VD, e9); NUM = res - rd (fused eviction)
            for h in range(2):
                pts = [psum.tile([128, 4, 96], F32, name=f"pb{c}", tag=f"pB{c}", bufs=1)
                       for c in range(6)]
                for k in range(9):
                    di, dj = divmod(k, 3)
                    for c in range(6):
                        r = 4 * (6 * h + c)
                        nc.tensor.matmul(
                            pts[c][:, :, :], dg(18 + k),
                            VD52[:, di + r: di + r + 4, dj: dj + 96],
                            start=(k == 0), stop=(k == 8))
                for c in range(6):
                    r = 4 * (6 * h + c)
                    nc.vector.tensor_sub(NUM[:, r: r + 4, :], pts[c][:, :, :],
                                         RD[:, r: r + 4, :])

            # ---- y1: 9-neighbor sigmoid diffusion (fp16 chain) ----
            vdc = VD52[:, 1:49, 1:97]
            first = True
            for k in range(9):
                di, dj = divmod(k, 3)
                sv = V16[:, di + 1: di + 49, dj: dj + 96]
                if k == 4:
                    # d == 0 -> sigmoid == 0.5, folded into tab (0.5 * w9_4)
                    tc4 = pool.tile([128, 48, 96], F16, tag="TA", bufs=3)
                    nc.vector.tensor_scalar_mul(tc4[:, :, :], sv, TAB[:, 4:5])
                    nc.vector.tensor_add(Y1[:, :, :], Y1[:, :, :], tc4[:, :, :])
                    continue
                svd = VD52[:, di: di + 48, dj: dj + 96]
                td = pool.tile([128, 48, 96], F16, tag="TD", bufs=3)
                nc.vector.tensor_sub(td[:, :, :], svd, vdc)
                # |x| = max(-x, x); TS then TT beats the 1x-only fused STT
                ta = pool.tile([128, 48, 96], F16, tag="TA", bufs=3)
                nc.vector.tensor_scalar_mul(ta[:, :, :], td[:, :, :], -1.0)
                nc.vector.tensor_tensor(td[:, :, :], td[:, :, :], ta[:, :, :],
                                        op=AluOpType.max)
                nc.scalar.activation(td[:, :, :], td[:, :, :], AF.Sigmoid)
                nc.vector.tensor_scalar_mul(td[:, :, :], td[:, :, :],
                                             TAB[:, k:k + 1])
                if first:
                    nc.vector.tensor_mul(Y1[:, :, :], td[:, :, :], sv)
                    first = False
                else:
                    nc.vector.tensor_mul(td[:, :, :], td[:, :, :], sv)
                    nc.vector.tensor_add(Y1[:, :, :], Y1[:, :, :], td[:, :, :])

            # ---- finale ----
            nc.vector.tensor_mul(Y2[:, :, :], NUM[:, :, :], RCP[:, :, :])
            # yt_int (rows plane 0..47 -> YT rows 1..49): o1 = 2*yt - y1 in place
            nc.vector.scalar_tensor_tensor(
                out=YT[:, 1:49, :], in0=YT[:, 1:49, :], scalar=2.0, in1=Y1[:, :, :],
                op0=AluOpType.mult, op1=AluOpType.subtract)
            nc.vector.tensor_sub(YO[:, :, :], YT[:, 1:49, :], Y2[:, :, :])
            nc.sync.dma_start(out=y_out[:, 0:HH, :], in_=YO[0:64, :, :])
            nc.sync.dma_start(out=y_out[:, HH:H, :], in_=YO[64:128, :, :])
    return nc


def _get_nc(name):
    if name not in _BUILD_CACHE:
        _BUILD_CACHE[name] = _build_gap_nc() if name == "gap" else _build_main_nc()
    return _BUILD_CACHE[name]


# --------------------------------------------------------------------------- #
# entry point
# --------------------------------------------------------------------------- #

def kernel(query, value, hard_sigmoid_masks, w_dp, b_dp, bn_g, bn_b, bn_rm, bn_rv,
           w_dc, b_dc, w_lin, b_lin, w_delin, b_delin):
    inputs = dict(query=query, value=value, w_dp=w_dp, b_dp=b_dp, bn_g=bn_g,
                  bn_b=bn_b, bn_rm=bn_rm, bn_rv=bn_rv, w_dc=w_dc, b_dc=b_dc,
                  w_lin=w_lin, b_lin=b_lin, w_delin=w_delin, b_delin=b_delin)
    value = np.ascontiguousarray(np.asarray(value, np.float32))
    planes = value.reshape(NPLANES, H, W)
    slices = [np.ascontiguousarray(planes[i * PPC:(i + 1) * PPC])
              for i in range(N_CORES)]
    core_ids = list(range(N_CORES))

    def _run(nc, in_maps):
        # the axon terminal occasionally wedges transiently after an earlier
        # fault; retry with backoff
        import time
        last = None
        for attempt in range(4):
            try:
                return run_bass_kernel_spmd(nc, in_maps, core_ids)
            except Exception as e:  # noqa: BLE001
                last = e
                time.sleep(20 * (attempt + 1))
        raise last

    # launch A: per-plane sums
    res = _run(_get_nc("gap"), [{"xv": s} for s in slices])
    gsum = np.concatenate([r["gsum"][:PPC, 0] + r["gsum"][PPC:, 0]
                           for r in res.results])          # (512,)
    gap = (gsum / (H * W)).reshape(B, C)

    # host: filter scalars + q head
    f9, w9, e9, qout = _gate_chain_host(inputs, gap)
    ae9 = np.abs(e9)
    wd = e9.reshape(NPLANES, 3, 3).sum(axis=1)             # (512,3) col sums
    w9h = w9.copy()
    w9h[:, 4] *= 0.5

    in_maps = []
    for i in range(N_CORES):
        sl = slice(i * PPC, (i + 1) * PPC)
        coef = np.concatenate([f9[sl], ae9[sl], e9[sl], wd[sl]], axis=1)  # (64,30)
        dg = np.zeros((N_DIAG, 128, 128), np.float16)
        idx = np.arange(128)
        dg[:, idx, idx] = coef.T[:, idx % PPC]
        tab = np.zeros((128, TAB_COLS), np.float32)
        tab[:, 0:9] = w9h[sl][idx % PPC]
        in_maps.append({"xv": slices[i], "diags": dg, "tab": tab})

    res = _run(_get_nc("main"), in_maps)
    y = np.concatenate([r["y"] for r in res.results]).reshape(B, C, H, W)
    return qout, y


if __name__ == "__main__":
    rng = np.random.default_rng(0)
    pass
